# revision 20
# baseline (speedup 1.0000x reference)
"""Trainium2 Bass kernel for nn_DMGHAN: input-proj -> Mamba block -> pooled
multi-granularity head. Data-parallel over batch: 8 samples -> 8 NeuronCores.

Device computes everything through the selective scan + gated time-mean
(99.96% of FLOPs); the tiny per-sample head epilogue (a few 256-vector
matvecs on the pooled feature) runs on the host during unsharding.

Self-contained: hardcodes all shapes; host-side prep transposes/folds weights.
"""
import numpy as np
from contextlib import ExitStack

# fixed architecture
B, L, EMBED = 8, 2048, 1024
DM, DI, N, DTR = 256, 512, 16, 16
NCLS = [5, 30, 80, 200, 600, 1500]
NCORES = 8
Q = 512          # t-quarter: pipeline chunk == scan quarter
NQ = L // Q

_PROG_CACHE = {}


def _build_program(debug_outs=False):
    import concourse.bass as bass
    import concourse.tile as tile
    from concourse import bacc, mybir

    F32 = mybir.dt.float32
    F32R = mybir.dt.float32r
    BF16 = mybir.dt.bfloat16
    AF = mybir.ActivationFunctionType
    OP = mybir.AluOpType

    nc = bacc.Bacc("TRN2", target_bir_lowering=False, debug=False,
                   num_devices=NCORES)

    def din(name, shape, dtype=None):
        return nc.dram_tensor(name, list(shape), dtype or F32,
                              kind="ExternalInput").ap()

    def dout(name, shape):
        return nc.dram_tensor(name, list(shape), F32,
                              kind="ExternalOutput").ap()

    xT = din("xT", (EMBED, L), F32R)
    WprojT = din("WprojT", (EMBED, DM), F32R)
    bproj = din("bproj", (2, 128, 1))
    Wu = din("Wu", (4, DM, DI), F32R)      # (W_in_u * conv_w[k]).T per tap
    convb = din("convb", (4, 128, 1))
    WzT = din("WzT", (DM, DI), F32R)
    WxpT = din("WxpT", (DI, DTR + 2 * N), F32R)
    WdtT = din("WdtT", (DTR, DI), F32R)
    bdt = din("bdt", (4, 128, 1))
    Ascale = din("Ascale", (4, 128, N))    # -exp(A_log), split by d-block
    DpDiag = din("DpDiag", (4, 128, 128), F32R)
    Ident = din("Ident", (128, 128), BF16)

    ymparts = dout("ymparts", (128, NQ * 4))   # column q*4+dblk
    if debug_outs:
        dbg_dt = dout("dbg_dt", (DI, L))
        dbg_u = dout("dbg_u", (DI, L))
        dbg_xdbc = dout("dbg_xdbc", (DTR + 2 * N, L))
        dbg_y = dout("dbg_y", (DI, L))

    bcrows_b = nc.dram_tensor("bcrows_b_internal", [N, L], BF16).ap()
    bcrows_c = nc.dram_tensor("bcrows_c_internal", [N, L], F32R).ap()

    with tile.TileContext(nc) as tc, ExitStack() as ctx:
        consts = ctx.enter_context(tc.tile_pool(name="consts", bufs=1))
        big = ctx.enter_context(tc.tile_pool(name="big", bufs=1))
        h0pool = ctx.enter_context(tc.tile_pool(name="h0p", bufs=1))
        wts = ctx.enter_context(tc.tile_pool(name="wts", bufs=1))
        # recycling pools for per-quarter intermediates
        upool = ctx.enter_context(tc.tile_pool(name="up", bufs=8))
        dtpool = ctx.enter_context(tc.tile_pool(name="dtp", bufs=8))
        wpool_ = ctx.enter_context(tc.tile_pool(name="wp_", bufs=8))
        sgpool = ctx.enter_context(tc.tile_pool(name="sgp", bufs=8))
        xdpool = ctx.enter_context(tc.tile_pool(name="xdp", bufs=2))
        xpool = ctx.enter_context(tc.tile_pool(name="xTp", bufs=12))
        apool = ctx.enter_context(tc.tile_pool(name="abh", bufs=6))
        bpool = ctx.enter_context(tc.tile_pool(name="bbh", bufs=6))
        hpool = ctx.enter_context(tc.tile_pool(name="hbh", bufs=4))
        hcpool = ctx.enter_context(tc.tile_pool(name="hcb", bufs=4))
        bcpool = ctx.enter_context(tc.tile_pool(name="bcast", bufs=4))
        drpool = ctx.enter_context(tc.tile_pool(name="drain", bufs=3))
        ps1 = ctx.enter_context(tc.tile_pool(name="ps1", bufs=1, space="PSUM"))
        ps2 = ctx.enter_context(tc.tile_pool(name="ps2", bufs=1, space="PSUM"))
        ps3 = ctx.enter_context(tc.tile_pool(name="ps3", bufs=1, space="PSUM"))
        psy = ctx.enter_context(tc.tile_pool(name="psy", bufs=1, space="PSUM"))

        h0T = [h0pool.tile([128, L], F32R, tag=f"h0T{m}", name=f"h0T{m}")
               for m in range(2)]

        def load_const(name, src, shape, dtype=None):
            t = consts.tile(list(shape), dtype or F32, tag=name, name=name)
            nc.sync.dma_start(t[:], src)
            return t

        bproj_t = [load_const(f"bproj{m}", bproj[m], (128, 1))
                   for m in range(2)]
        convb_t = [load_const(f"convb{m}", convb[m], (128, 1))
                   for m in range(4)]
        bdt_t = [load_const(f"bdt{m}", bdt[m], (128, 1)) for m in range(4)]
        Asc_t = [load_const(f"Asc{m}", Ascale[m], (128, N)) for m in range(4)]
        dpd_t = [load_const(f"dpd{m}", DpDiag[m], (128, 128), F32R)
                 for m in range(4)]
        id16 = load_const("id16", Ident[:], (128, 128), BF16)
        ymp = big.tile([128, NQ * 4], F32, tag="ymp", name="ymp")
        states = big.tile([128, 4 * N], F32, tag="states", name="states")

        def act(out_ap, in_ap, func, bias=0.0, scale=1.0):
            nc.scalar.activation(out_ap, in_ap, func, bias=bias, scale=scale)

        # ---- weights (loaded up front; DMA overlaps with x chunk loads) ----
        wp = []
        for e in range(8):
            t = wts.tile([128, DM], F32R, tag=f"wp{e}", name=f"wp{e}")
            nc.sync.dma_start(t[:], WprojT[e * 128:(e + 1) * 128, :])
            wp.append(t)
        wut = []
        for k in range(4):
            row = []
            for kb in range(2):
                t = wts.tile([128, DI], F32R, tag=f"wu{k}_{kb}",
                             name=f"wu{k}_{kb}")
                nc.sync.dma_start(t[:], Wu[k, kb * 128:(kb + 1) * 128, :])
                row.append(t)
            wut.append(row)
        wxt = []
        for kb in range(4):
            t = wts.tile([128, DTR + 2 * N], F32R, tag=f"wxp{kb}",
                         name=f"wxp{kb}")
            nc.sync.dma_start(t[:], WxpT[kb * 128:(kb + 1) * 128, :])
            wxt.append(t)
        wdt_t = wts.tile([DTR, DI], F32R, tag="wdt", name="wdt")
        nc.sync.dma_start(wdt_t[:], WdtT[:])
        wzt = []
        for kb in range(2):
            t = wts.tile([128, DI], F32R, tag=f"wz{kb}", name=f"wz{kb}")
            nc.sync.dma_start(t[:], WzT[kb * 128:(kb + 1) * 128, :])
            wzt.append(t)

        def chunk_pipeline(fq):
            """x chunk -> h0 -> u -> xdbc -> dt, w, sg for t-range fq*Q.."""
            c0 = fq * Q
            xc = []
            for e in range(8):
                t = xpool.tile([128, Q], F32R, tag="xc", name=f"xc{e}_{fq}")
                nc.sync.dma_start(t[:], xT[e * 128:(e + 1) * 128, c0:c0 + Q])
                xc.append(t)
            for mt in range(2):
                ps = ps1.tile([128, Q], F32, tag="ps1", name="ps1")
                for kb in range(8):
                    nc.tensor.matmul(ps[:], wp[kb][:, mt * 128:(mt + 1) * 128],
                                     xc[kb][:], start=(kb == 0),
                                     stop=(kb == 7))
                act(h0T[mt][:, c0:c0 + Q], ps[:], AF.Identity,
                    bias=bproj_t[mt][:])
            uq, dtq, wq, sgq = [], [], [], []
            for mt in range(4):
                ms = slice(mt * 128, (mt + 1) * 128)
                ps = ps2.tile([128, Q], F32, tag="ps2", name="ps2")
                first = True
                for k in (3, 2, 1, 0):
                    s = 3 - k
                    for kb in range(2):
                        if c0 == 0 and s > 0:
                            # odd-offset edge: fp32r alignment rules forbid
                            # it; run these few in plain fp32
                            nc.tensor.matmul(
                                ps[:, s:Q].bitcast(F32),
                                wut[k][kb][:, ms].bitcast(F32),
                                h0T[kb][:, 0:Q - s].bitcast(F32),
                                start=first, stop=(k == 0 and kb == 1),
                                skip_group_check=True)
                        else:
                            nc.tensor.matmul(
                                ps[:], wut[k][kb][:, ms],
                                h0T[kb][:, c0 - s:c0 - s + Q],
                                start=first, stop=(k == 0 and kb == 1),
                                skip_group_check=True)
                        first = False
                ut = upool.tile([128, Q], F32R, tag="u", name=f"u{mt}_{fq}")
                act(ut[:], ps[:], AF.Silu, bias=convb_t[mt][:])
                uq.append(ut)
            for mt in range(4):
                ms = slice(mt * 128, (mt + 1) * 128)
                ps = ps1.tile([128, Q], F32, tag="ps1", name="psz")
                for kb in range(2):
                    nc.tensor.matmul(ps[:], wzt[kb][:, ms],
                                     h0T[kb][:, c0:c0 + Q],
                                     start=(kb == 0), stop=(kb == 1))
                sgt = sgpool.tile([128, Q], BF16, tag="sg", name=f"sg{mt}_{fq}")
                act(sgt[:], ps[:], AF.Silu)
                sgq.append(sgt)
            xdbc = xdpool.tile([DTR + 2 * N, Q], F32R, tag="xdbc",
                               name=f"xdbc{fq}")
            ps = ps3.tile([DTR + 2 * N, Q], F32, tag="ps3", name="ps3")
            for kb in range(4):
                nc.tensor.matmul(ps[:], wxt[kb][:], uq[kb][:],
                                 start=(kb == 0), stop=(kb == 3))
            act(xdbc[:], ps[:], AF.Copy)
            nc.gpsimd.dma_start(bcrows_b[:, c0:c0 + Q],
                                xdbc[DTR:DTR + N, :])
            nc.sync.dma_start(bcrows_c[:, c0:c0 + Q], xdbc[DTR + N:, :])
            for mt in range(4):
                ms = slice(mt * 128, (mt + 1) * 128)
                ps = ps3.tile([128, Q], F32, tag="ps3b", name="ps3b")
                nc.tensor.matmul(ps[:], wdt_t[:, ms], xdbc[0:DTR, :],
                                 start=True, stop=True)
                # softplus(x + b) = Ln(Exp(x + b) + 1) (no Softplus table)
                spt = consts.tile([128, Q], F32, tag="spt", name="spt",
                                  bufs=2)
                act(spt[:], ps[:], AF.Exp, bias=bdt_t[mt][:])
                dtt = dtpool.tile([128, Q], F32, tag="dt", name=f"dt{mt}_{fq}")
                act(dtt[:], spt[:], AF.Ln, bias=1.0)
                dtq.append(dtt)
                wt = wpool_.tile([128, Q], BF16, tag="w", name=f"w{mt}_{fq}")
                nc.vector.tensor_mul(wt[:], dtt[:], uq[mt][:])
                wq.append(wt)
            if debug_outs:
                nc.gpsimd.dma_start(dbg_xdbc[:, c0:c0 + Q], xdbc[:])
                for mt in range(4):
                    ms = slice(mt * 128, (mt + 1) * 128)
                    nc.sync.dma_start(dbg_dt[ms, c0:c0 + Q], dtq[mt][:])
                    nc.gpsimd.dma_start(dbg_u[ms, c0:c0 + Q], uq[mt][:])
            return uq, dtq, wq, sgq

        def scan_quarter(q, uq, dtq, wq, sgq):
            c0 = q * Q
            yps = []
            for dblk in range(4):
                ps = psy.tile([128, Q], F32, tag=f"yps{dblk}",
                              name=f"yps{dblk}")
                nc.tensor.matmul(ps[:], dpd_t[dblk][:], uq[dblk][:],
                                 start=True, stop=False,
                                 skip_group_check=True)
                yps.append(ps)
            for n in range(N):
                Bb = bcpool.tile([128, Q], BF16, tag="Bb", name="Bb")
                Cb = bcpool.tile([128, Q], F32R, tag="Cb", name="Cb")
                brow = bcrows_b[n:n + 1, c0:c0 + Q]
                crow = bcrows_c[n:n + 1, c0:c0 + Q]
                nc.sync.dma_start(
                    Bb[:], bass.AP(tensor=brow.tensor, offset=brow.offset,
                                   ap=[[0, 128]] + list(brow.ap[1:])))
                nc.sync.dma_start(
                    Cb[:], bass.AP(tensor=crow.tensor, offset=crow.offset,
                                   ap=[[0, 128]] + list(crow.ap[1:])))
                for dblk in range(4):
                    scol = dblk * N + n
                    a_t = apool.tile([128, Q], BF16, tag="a", name="a")
                    act(a_t[:], dtq[dblk][:], AF.Exp,
                        scale=Asc_t[dblk][:, n:n + 1])
                    b_t = bpool.tile([128, Q], BF16, tag="b", name="b")
                    nc.vector.tensor_mul(b_t[:], wq[dblk][:], Bb[:])
                    h_t = hpool.tile([128, Q], F32, tag="h", name="h")
                    ini = 0.0 if q == 0 else states[:, scol:scol + 1]
                    nc.vector.tensor_tensor_scan(
                        h_t[:], a_t[:], b_t[:], ini, OP.mult, OP.add)
                    if q < NQ - 1:
                        act(states[:, scol:scol + 1], h_t[:, Q - 1:Q],
                            AF.Copy)
                    hc_t = hcpool.tile([128, Q], BF16, tag="hc", name="hc")
                    nc.vector.tensor_mul(hc_t[:], h_t[:], Cb[:])
                    nc.tensor.matmul(yps[dblk][:], id16[:], hc_t[:],
                                     start=False, stop=(n == N - 1),
                                     skip_group_check=True)
            for dblk in range(4):
                dr = drpool.tile([128, Q], BF16, tag="dr", name="dr")
                act(dr[:], yps[dblk][:], AF.Copy)
                yg = drpool.tile([128, Q], BF16, tag="yg", name="yg")
                nc.vector.tensor_mul(yg[:], dr[:], sgq[dblk][:])
                nc.vector.tensor_reduce(
                    ymp[:, q * 4 + dblk:q * 4 + dblk + 1], yg[:],
                    mybir.AxisListType.X, OP.add)
                if debug_outs:
                    nc.sync.dma_start(
                        dbg_y[dblk * 128:(dblk + 1) * 128, c0:c0 + Q], yg[:])

        for fq in range(NQ):
            args = chunk_pipeline(fq)
            scan_quarter(fq, *args)

        nc.sync.dma_start(ymparts[:], ymp[:])

    nc.compile()
    return nc


def _get_program(debug_outs=False):
    key = ("prog", debug_outs)
    if key not in _PROG_CACHE:
        _PROG_CACHE[key] = _build_program(debug_outs)
    return _PROG_CACHE[key]


def _host_prep(inputs):
    """Build the per-core input maps from the full problem inputs."""
    f32 = np.float32

    def c(a):
        return np.ascontiguousarray(np.asarray(a, dtype=f32))

    x = c(inputs["x"])
    W_proj = c(inputs["W_proj"]); b_proj = c(inputs["b_proj"])
    W_in = c(inputs["W_in"]); conv_w = c(inputs["conv_w"])
    conv_b = c(inputs["conv_b"]); W_xp = c(inputs["W_xp"])
    W_dt = c(inputs["W_dt"]); b_dt = c(inputs["b_dt"])
    A_log = c(inputs["A_log"]); Dp = c(inputs["Dp"])

    Wu = np.stack([
        c((conv_w[:, 0, k][:, None] * W_in[:DI]).T) for k in range(4)
    ])  # (4, 256, 512)
    shared = {
        "WprojT": c(W_proj.T),
        "bproj": b_proj.reshape(2, 128, 1),
        "Wu": Wu,
        "convb": conv_b.reshape(4, 128, 1),
        "WzT": c(W_in[DI:].T),
        "WxpT": c(W_xp.T),
        "WdtT": c(W_dt.T),
        "bdt": b_dt.reshape(4, 128, 1),
        "Ascale": c(-np.exp(A_log)).reshape(4, 128, N),
        "DpDiag": np.stack([np.diag(Dp[k * 128:(k + 1) * 128])
                            for k in range(4)]).astype(f32),
        "Ident": np.eye(128, dtype=np.float32),
    }
    in_maps = []
    for core in range(NCORES):
        m = dict(shared)
        m["xT"] = c(x[core].T)
        in_maps.append(m)
    return in_maps


def _host_head(pooled_rows, inputs):
    """The seq_len=1 attention/gate head + classifiers (tiny, fp32 numpy).

    pooled_rows: (B, 256) pooled features from the device.
    """
    f32 = np.float32
    mha_in_w = np.asarray(inputs["mha_in_w"], f32)
    mha_in_b = np.asarray(inputs["mha_in_b"], f32)
    mha_out_w = np.asarray(inputs["mha_out_w"], f32)
    mha_out_b = np.asarray(inputs["mha_out_b"], f32)
    gate_w = np.asarray(inputs["gate_w"], f32)
    gate_b = np.asarray(inputs["gate_b"], f32)

    pooled = pooled_rows
    feats = []
    cur = None
    for i in range(6):
        if i == 0:
            f = pooled
        else:
            g = 1.0 / (1.0 + np.exp(-(np.concatenate([cur, pooled], -1)
                                      @ gate_w[i - 1].T + gate_b[i - 1])))
            f = (g * cur + (1.0 - g) * pooled).astype(f32)
        v = f @ mha_in_w[i][2 * DM:].T + mha_in_b[i][2 * DM:]
        cur = (v @ mha_out_w[i].T + mha_out_b[i]).astype(f32)
        feats.append(cur)

    outs = []
    for i in range(6):
        Wc = np.asarray(inputs[f"Wc{i}"], f32)
        bc = np.asarray(inputs[f"bc{i}"], f32)
        outs.append((feats[i] @ Wc.T + bc).astype(f32))
    return tuple(outs)


def _finish(res, inputs):
    W_out = np.asarray(inputs["W_out"], np.float32)
    pooled_rows = []
    for core in range(NCORES):
        p = res.results[core]["ymparts"]          # (128, NQ*4)
        ymean = sum(p[:, 4 * qq:4 * qq + 4] for qq in range(NQ))
        yvec = ymean.T.reshape(DI)                # d = dblk*128 + p
        pooled_rows.append((W_out @ yvec) / np.float32(L))
    return _host_head(np.stack(pooled_rows).astype(np.float32), inputs)


def kernel(**inputs):
    from concourse.bass_utils import run_bass_kernel_spmd

    nc = _get_program()
    in_maps = _host_prep(inputs)
    res = run_bass_kernel_spmd(nc, in_maps, list(range(NCORES)))
    return _finish(res, inputs)
